# revision 14
# baseline (speedup 1.0000x reference)
"""Trainium2 Bass kernel for nn_CrossAttention (B=2, N=2048, D=768, H=12).

Sharding: (batch, head-group) across 8 cores — core c handles batch c//4 and
heads [3g, 3g+2] where g = c%4. Attention fully local per (batch, head).

v2 design (bf16 operands, fp32 PSUM accumulation):
  - All inputs DMA'd as bf16 (host converts); no f32->f32r staging copies.
  - S^T matmuls contract over pd=64 only, so they run as ROW-TILED pairs on
    the PE (tile_position (0,0)/(64,0)): heads 0,1 pair naturally (their q/k
    live on partition halves of one [128,N] tile); head 2's projections are
    emitted with duplicated weight columns so its q/k are replicated across
    both halves, letting adjacent j-tiles pair.
  - exp at [128,1024] granularity (one ACT call spans the 2 PSUM banks the
    row-tiled S pair writes). ACT is the bottleneck engine (~110us).
  - AV ("v'" with a ones column per head) accumulates [65, 512] per i-chunk
    in PSUM over all 16 j-tiles; row 64 is the softmax denominator.
  - Software pipeline: k/v/q projections are woven between attention slots,
    so exp starts a few us in; DMA/projections hide under the ACT stream.
  - PSUM budget: S pairs 2x[128,1024] (4 banks) + AV accum 3x[128,512]
    + projections 1x[128,512] = 8 banks.
"""

import sys

if "/opt/trn_rl_repo" not in sys.path:
    sys.path.insert(0, "/opt/trn_rl_repo")

import numpy as np
import ml_dtypes

import concourse.bass as bass
import concourse.tile as tile
from concourse import bacc, mybir
from concourse.bass_utils import run_bass_kernel_spmd

F32 = mybir.dt.float32
BF16 = mybir.dt.bfloat16
AF = mybir.ActivationFunctionType
BF_NP = np.dtype(ml_dtypes.bfloat16)

B, N, D, H, PD = 2, 2048, 768, 12, 64
KC = 6  # contraction chunks: 768 / 128
NI = 4  # i (query) chunks of 512
NJ = 16  # j (key) chunks of 128

# test harness hooks
TRACE = False
LAST_RESULTS = None

_cache: dict = {}


def _emit(tc, t, loop_iters=1):
    if loop_iters > 1:
        with tc.For_i(0, loop_iters, 1):
            _emit_body(tc, t)
    else:
        _emit_body(tc, t)


def _emit_body(tc, t):
    nc = tc.nc
    import contextlib

    with contextlib.ExitStack() as ctx:
        persist = ctx.enter_context(tc.tile_pool(name="persist", bufs=1))
        expp = ctx.enter_context(tc.tile_pool(name="expp", bufs=3))
        outp = ctx.enter_context(tc.tile_pool(name="outp", bufs=2))
        smallp = ctx.enter_context(tc.tile_pool(name="smallp", bufs=2))
        ps_s = ctx.enter_context(tc.tile_pool(name="ps_s", bufs=2, space="PSUM"))
        ps_o = ctx.enter_context(tc.tile_pool(name="ps_o", bufs=3, space="PSUM"))
        ps_p = ctx.enter_context(tc.tile_pool(name="ps_p", bufs=1, space="PSUM"))

        # ---- persistent SBUF ----
        wq01 = persist.tile([128, KC, 128], BF16)
        wq2d = persist.tile([128, KC, 128], BF16)
        wk01 = persist.tile([128, KC, 128], BF16)
        wk2d = persist.tile([128, KC, 128], BF16)
        wv = persist.tile([128, KC, 192], BF16)
        xq_sb = persist.tile([128, KC, N], BF16)
        xkv_sb = persist.tile([128, KC, N], BF16)
        qT01 = persist.tile([128, N], BF16)
        qT2d = persist.tile([128, N], BF16)
        kT01 = persist.tile([128, N], BF16)
        kT2d = persist.tile([128, N], BF16)
        v01 = persist.tile([128, NJ, 130], BF16)  # [64 v_h0 | 1 | 64 v_h1 | 1]
        v2 = persist.tile([128, NJ, 65], BF16)  # [64 v_h2 | 1]
        b4 = persist.tile([128, 4], F32)  # bq01 | bq2d | bk01 | bk2d
        bv_sb = persist.tile([1, 192], BF16)
        ones_row = persist.tile([1, 128], BF16)

        # ---- prologue DMAs (queue order matters: first-needed first) ----
        def wview(name):
            return t[name].rearrange("(kc p) m -> p kc m", p=128)

        # host pre-packs weights in SBUF layout [128, kc*w] (1.5-2.3KB HBM
        # lines) and x as [chunk][128][kc*512] (6KB contiguous per partition
        # per chunk); the first x chunk bootstraps the projections.
        def xchunk(name, c):
            return t[name][c].rearrange("p (kc n) -> p kc n", n=512)

        # q01's deps first (wq01, b4, xq chunk 0), then k01's, then v's
        nc.sync.dma_start(wq01[:], t["wq01"].rearrange("p (kc m) -> p kc m", kc=KC))
        nc.sync.dma_start(b4[:], t["b4"])
        nc.sync.dma_start(xq_sb[:, :, 0:512], xchunk("xq", 0))
        nc.sync.dma_start(wk01[:], t["wk01"].rearrange("p (kc m) -> p kc m", kc=KC))
        nc.sync.dma_start(xkv_sb[:, :, 0:512], xchunk("xkv", 0))
        nc.sync.dma_start(wv[:], t["wv"].rearrange("p (kc m) -> p kc m", kc=KC))
        nc.sync.dma_start(bv_sb[:], t["bv"])
        for c in range(1, 4):
            sl = slice(c * 512, (c + 1) * 512)
            nc.sync.dma_start(xkv_sb[:, :, sl], xchunk("xkv", c))
        nc.sync.dma_start(wq2d[:], t["wq2d"].rearrange("p (kc m) -> p kc m", kc=KC))
        nc.sync.dma_start(wk2d[:], t["wk2d"].rearrange("p (kc m) -> p kc m", kc=KC))
        for c in range(1, 4):
            sl = slice(c * 512, (c + 1) * 512)
            nc.sync.dma_start(xq_sb[:, :, sl], xchunk("xq", c))

        # ones columns of v' (constant) + ones row for bias matmuls
        nc.vector.memset(ones_row[:], 1.0)
        nc.vector.memset(v01[:, :, 64:65], 1.0)
        nc.vector.memset(v01[:, :, 129:130], 1.0)
        nc.vector.memset(v2[:, :, 64:65], 1.0)

        # ---- projection emitters (each a closure; woven into the stream) ----
        def proj_q(w_sb, dst, bias_col, iq):
            ps = ps_p.tile([128, 512], F32, tag="pp")
            sl = slice(iq * 512, (iq + 1) * 512)
            for kc in range(KC):
                nc.tensor.matmul(
                    ps[:],
                    w_sb[:, kc, :],
                    xq_sb[:, kc, sl],
                    start=(kc == 0),
                    stop=(kc == KC - 1),
                )
            nc.vector.tensor_scalar_add(dst[:, sl], ps[:], b4[:, bias_col : bias_col + 1])

        def proj_k(w_sb, dst, bias_col, jp):
            ps = ps_p.tile([128, 256], F32, tag="pp")
            sl = slice(jp * 256, (jp + 1) * 256)
            for kc in range(KC):
                nc.tensor.matmul(
                    ps[:],
                    w_sb[:, kc, :],
                    xkv_sb[:, kc, sl],
                    start=(kc == 0),
                    stop=(kc == KC - 1),
                )
            nc.vector.tensor_scalar_add(dst[:, sl], ps[:], b4[:, bias_col : bias_col + 1])

        def proj_v(jc):
            ps = ps_p.tile([128, 192], F32, tag="pp")
            sl = slice(jc * 128, (jc + 1) * 128)
            for kc in range(KC):
                nc.tensor.matmul(
                    ps[:],
                    xkv_sb[:, kc, sl],
                    wv[:, kc, :],
                    start=(kc == 0),
                    stop=False,
                )
            nc.tensor.matmul(ps[:], ones_row[:], bv_sb[:], start=False, stop=True)
            src01 = ps[:, 0:128].rearrange("p (h c) -> p h c", h=2)
            dst01 = v01[:, jc : jc + 1, :].rearrange(
                "p j (h c) -> p (j h) c", c=65
            )[:, :, 0:64]
            nc.vector.tensor_copy(dst01, src01)
            nc.vector.tensor_copy(v2[:, jc, 0:64], ps[:, 128:192])

        # background job list (no ordering constraint vs h01 attention),
        # popped one per slot from pass iq=1 onward
        jobs = []
        jobs += [lambda jp=jp: proj_k(wk2d, kT2d, 3, jp) for jp in range(8)]
        jobs += [lambda iq=iq: proj_q(wq2d, qT2d, 1, iq) for iq in range(4)]

        def pop_jobs(k):
            for _ in range(k):
                if jobs:
                    jobs.pop(0)()

        # prologue projections: minimum to start attention
        proj_q(wq01, qT01, 0, 0)
        proj_k(wk01, kT01, 2, 0)
        proj_v(0)
        proj_v(1)

        def out_stage(po, h, iq):
            recip = smallp.tile([1, 512], F32, tag="recip")
            nc.vector.reciprocal(recip[:], po[64:65, :])
            bcast = smallp.tile([64, 512], F32, tag="bcast")
            nc.gpsimd.partition_broadcast(bcast[:], recip[:])
            out_sb = outp.tile([64, 512], F32, tag="out")
            nc.vector.tensor_mul(out_sb[:], po[0:64, :], bcast[:])
            nc.sync.dma_start(t["o_t"][h, :, iq * 512 : (iq + 1) * 512], out_sb[:])

        # ---- heads 0,1: row-tiled S pairs (h0 on rows 0:64, h1 on 64:128) ----
        for iq in range(NI):
            isl = slice(iq * 512, (iq + 1) * 512)
            po0 = ps_o.tile([128, 512], F32, tag="po", name=f"po0_{iq}")
            po1 = ps_o.tile([128, 512], F32, tag="po", name=f"po1_{iq}")
            for jc in range(NJ):
                jsl = slice(jc * 128, (jc + 1) * 128)
                pss = ps_s.tile([128, 1024], F32, tag="pss")
                nc.tensor.matmul(
                    pss[:, 0:512], kT01[0:64, jsl], qT01[0:64, isl],
                    start=True, stop=True, tile_position=(0, 0),
                )
                nc.tensor.matmul(
                    pss[:, 512:1024], kT01[64:128, jsl], qT01[64:128, isl],
                    start=True, stop=True, tile_position=(64, 0),
                )
                ex = expp.tile([128, 1024], BF16, tag="ex")
                nc.scalar.activation(ex[:], pss[:], AF.Exp)
                nc.tensor.matmul(
                    po0[0:65, :], v01[:, jc, 0:65], ex[:, 0:512],
                    start=(jc == 0), stop=(jc == NJ - 1),
                )
                nc.tensor.matmul(
                    po1[0:65, :], v01[:, jc, 65:130], ex[:, 512:1024],
                    start=(jc == 0), stop=(jc == NJ - 1),
                )
                # deterministic weave: producers stay >=1 slot ahead of use
                if iq == 0:
                    if jc + 2 < NJ:
                        proj_v(jc + 2)
                    nxt = jc + 2
                    if nxt % 2 == 0 and nxt // 2 < NJ // 2:
                        proj_k(wk01, kT01, 2, nxt // 2)
                elif jc % 2 == 0:
                    pop_jobs(1)
                if jc == 6 and iq < NI - 1:
                    proj_q(wq01, qT01, 0, iq + 1)
            out_stage(po0, 0, iq)
            out_stage(po1, 1, iq)

        # ---- head 2: row-tiled adjacent j-tile pairs (duplicated q/k) ----
        for iq in range(NI):
            isl = slice(iq * 512, (iq + 1) * 512)
            po2 = ps_o.tile([128, 512], F32, tag="po", name=f"po2_{iq}")
            for jp in range(NJ // 2):
                ja, jb = 2 * jp, 2 * jp + 1
                pss = ps_s.tile([128, 1024], F32, tag="pss")
                nc.tensor.matmul(
                    pss[:, 0:512],
                    kT2d[0:64, ja * 128 : (ja + 1) * 128], qT2d[0:64, isl],
                    start=True, stop=True, tile_position=(0, 0),
                )
                nc.tensor.matmul(
                    pss[:, 512:1024],
                    kT2d[64:128, jb * 128 : (jb + 1) * 128], qT2d[64:128, isl],
                    start=True, stop=True, tile_position=(64, 0),
                )
                ex = expp.tile([128, 1024], BF16, tag="ex")
                nc.scalar.activation(ex[:], pss[:], AF.Exp)
                nc.tensor.matmul(
                    po2[0:65, :], v2[:, ja, 0:65], ex[:, 0:512],
                    start=(jp == 0), stop=False,
                )
                nc.tensor.matmul(
                    po2[0:65, :], v2[:, jb, 0:65], ex[:, 512:1024],
                    start=False, stop=(jp == NJ // 2 - 1),
                )
                pop_jobs(2)
            out_stage(po2, 2, iq)

        pop_jobs(len(jobs))


def _build(loop_iters=1):
    key = ("nc", loop_iters)
    if key in _cache:
        return _cache[key]
    nc = bacc.Bacc("TRN2", target_bir_lowering=False, debug=False, num_devices=8)
    t = {}
    for name, shape, dt in [
        ("xq", [NI, 128, KC * 512], BF16),
        ("xkv", [NI, 128, KC * 512], BF16),
        ("wq01", [128, KC * 128], BF16),
        ("wq2d", [128, KC * 128], BF16),
        ("wk01", [128, KC * 128], BF16),
        ("wk2d", [128, KC * 128], BF16),
        ("wv", [128, KC * 192], BF16),
        ("b4", [128, 4], F32),
        ("bv", [1, 192], BF16),
    ]:
        t[name] = nc.dram_tensor(name, shape, dt, kind="ExternalInput").ap()
    t["o_t"] = nc.dram_tensor("o_t", [3, PD, N], F32, kind="ExternalOutput").ap()
    with tile.TileContext(nc) as tc:
        _emit(tc, t, loop_iters)
    nc.compile()
    _cache[key] = nc
    return nc


def _pack_x(xT):
    # [768, 2048] -> [4, 128, 6*512]: (kc p, c n') -> (c, p, kc n')
    r = xT.reshape(KC, 128, NI, 512)
    return np.ascontiguousarray(r.transpose(2, 1, 0, 3).reshape(NI, 128, KC * 512))


def _pack_w(wT):
    # [768, m] -> [128, 6*m]: (kc p, m) -> (p, kc m)
    m = wT.shape[1]
    r = wT.reshape(KC, 128, m)
    return np.ascontiguousarray(r.transpose(1, 0, 2).reshape(128, KC * m))


def _shard(x1, x2, Wq, bq, Wkv, bkv):
    in_maps = []
    for c in range(8):
        b, g = divmod(c, 4)
        h0 = 192 * g  # first row of this core's 3-head slice
        q01 = Wq[h0 : h0 + 128]
        q2 = Wq[h0 + 128 : h0 + 192]
        k01 = Wkv[h0 : h0 + 128]
        k2 = Wkv[h0 + 128 : h0 + 192]
        vw = Wkv[D + h0 : D + h0 + 192]
        b4 = np.stack(
            [
                bq[h0 : h0 + 128],
                np.tile(bq[h0 + 128 : h0 + 192], 2),
                bkv[h0 : h0 + 128],
                np.tile(bkv[h0 + 128 : h0 + 192], 2),
            ],
            axis=1,
        ).astype(np.float32)
        in_maps.append(
            {
                "xq": _pack_x(x2[b].T.astype(BF_NP)),
                "xkv": _pack_x(x1[b].T.astype(BF_NP)),
                "wq01": _pack_w(q01.T.astype(BF_NP)),
                "wq2d": _pack_w(np.concatenate([q2, q2]).T.astype(BF_NP)),
                "wk01": _pack_w(k01.T.astype(BF_NP)),
                "wk2d": _pack_w(np.concatenate([k2, k2]).T.astype(BF_NP)),
                "wv": _pack_w(vw.T.astype(BF_NP)),
                "b4": b4,
                "bv": bkv[D + h0 : D + h0 + 192].reshape(1, -1).astype(BF_NP),
            }
        )
    return in_maps


def kernel(x1, x2, Wq, bq, Wkv, bkv):
    global LAST_RESULTS
    x1 = np.asarray(x1, dtype=np.float32)
    x2 = np.asarray(x2, dtype=np.float32)
    Wq = np.asarray(Wq, dtype=np.float32)
    bq = np.asarray(bq, dtype=np.float32)
    Wkv = np.asarray(Wkv, dtype=np.float32)
    bkv = np.asarray(bkv, dtype=np.float32)

    nc = _build()
    in_maps = _shard(x1, x2, Wq, bq, Wkv, bkv)
    res = run_bass_kernel_spmd(nc, in_maps, core_ids=list(range(8)), trace=TRACE)
    LAST_RESULTS = res

    out = np.empty((B, H, N, PD), np.float32)
    for c in range(8):
        b, g = divmod(c, 4)
        ot = res.results[c]["o_t"]  # (3, 64, 2048)
        out[b, 3 * g : 3 * g + 3] = ot.transpose(0, 2, 1)
    return out.reshape(B, N, D)
